# revision 2
# baseline (speedup 1.0000x reference)
"""Trainium2 Bass kernel for nn_MultiHeadAttention_firstlayer.

Math notes (derived from the reference):
  - d_k == 1 and the key projection contributes kk[b,i,h] uniformly to every
    element of softmax row (h,b,i,:).  Softmax is shift-invariant, so the
    attention probabilities are exactly softmax(mask_diag(posi_att[h])) --
    independent of k, q, w_k, b_k AND of the batch index.
  - Therefore: attn[h*B+b] = A[h] = softmax_row(posi_att[h] masked at the
    diagonal), and out[b] = LN(concat_h(A[h] @ vv[b,:,h,:]) @ w_fc.T + b_fc).

Distribution (8 cores, no collectives):
  - Core c computes the full forward for batch b=c (all 16 heads), and
    additionally writes the normalized attention matrices for heads 2c, 2c+1.
  - All per-core variation is pushed into the input data via a head
    permutation (owned heads first), so one SPMD program serves all cores.

Layouts (host pre-transposes so every matmul contracts over partitions):
  - posiT[h, k, q] = posi_att[h, q, k]   (bf16)  -> exp gives E^T [k, q]
  - vT[dm, tok] = v[c].T                 (bf16)
  - wvT = w_v.T (col-permuted), wfcT = w_fc.T (row-permuted)   (bf16)
  - einsum:  pe[d, q]  = sum_k vv[k, d] * E^T[k, q]   (ones column gives the
    softmax denominator s[q] as psum row 64 for free)
  - fc:      pf[q, dm] = sum_hd outT[hd, q] * wfcT[hd, dm]
"""

import sys

sys.path.insert(0, "/opt/trn_rl_repo")

import numpy as np
import ml_dtypes

import concourse.mybir as mybir
import concourse.tile as tile
from concourse import bacc
from concourse.bass_utils import run_bass_kernel_spmd

BF = ml_dtypes.bfloat16
F32 = mybir.dt.float32
BF16 = mybir.dt.bfloat16
AF = mybir.ActivationFunctionType
ALU = mybir.AluOpType
AX = mybir.AxisListType

H, DV, DM, B, L = 16, 64, 1024, 8, 1024
NCORES = 8
LN_EPS = 1e-5
MASK = -30.0  # exp(-30) ~ 9e-14: negligible vs row sums ~1e3

_NC_CACHE = {}


def _build_nc():
    nc = bacc.Bacc("TRN2", target_bir_lowering=False, debug=False, num_devices=NCORES)

    posiT = nc.dram_tensor("posiT", [H, L, L], BF16, kind="ExternalInput")
    vT = nc.dram_tensor("vT", [DM, L], BF16, kind="ExternalInput")
    wvT = nc.dram_tensor("wvT", [DM, H * DV], BF16, kind="ExternalInput")
    wfcT = nc.dram_tensor("wfcT", [H * DV, DM], BF16, kind="ExternalInput")
    bv = nc.dram_tensor("bv", [1, H * DV], BF16, kind="ExternalInput")
    bfc = nc.dram_tensor("bfc", [1, DM], BF16, kind="ExternalInput")
    g_bc = nc.dram_tensor("g_bc", [128, DM], F32, kind="ExternalInput")
    b_bc = nc.dram_tensor("b_bc", [128, DM], F32, kind="ExternalInput")
    eye = nc.dram_tensor("eye", [128, 128], BF16, kind="ExternalInput")  # MASK * I
    ones_f = nc.dram_tensor("ones_f", [1, 128], F32, kind="ExternalInput")
    ones_b = nc.dram_tensor("ones_b", [1, 128], BF16, kind="ExternalInput")

    A_out = nc.dram_tensor("A_out", [2, L, L], F32, kind="ExternalOutput")
    y_out = nc.dram_tensor("y_out", [L, DM], F32, kind="ExternalOutput")

    with tile.TileContext(nc) as tc:
        with tc.tile_pool(name="const", bufs=1) as cp:
            wfc_sb = [cp.tile([128, DM], BF16, tag=f"wfc{i}", name=f"wfc{i}") for i in range(8)]
            vv_sb = [cp.tile([128, H * 65], BF16, tag=f"vv{i}", name=f"vv{i}") for i in range(8)]
            outT_sb = [cp.tile([128, L], BF16, tag=f"oT{i}", name=f"oT{i}") for i in range(8)]
            eye_sb = cp.tile([128, 128], BF16, tag="eye")
            onf_sb = cp.tile([1, 128], F32, tag="onf")
            onb_sb = cp.tile([1, 128], BF16, tag="onb")
            bv_sb = cp.tile([1, H * DV], BF16, tag="bv")
            bfc_sb = cp.tile([1, DM], BF16, tag="bfc")
            g_sb = cp.tile([128, DM], F32, tag="g")
            b_sb = cp.tile([128, DM], F32, tag="b")

            for i in range(8):
                nc.sync.dma_start(wfc_sb[i][:], wfcT.ap()[i * 128:(i + 1) * 128, :])
            nc.sync.dma_start(eye_sb[:], eye.ap())
            nc.sync.dma_start(onf_sb[:], ones_f.ap())
            nc.sync.dma_start(onb_sb[:], ones_b.ap())
            nc.sync.dma_start(bv_sb[:], bv.ap())
            nc.sync.dma_start(bfc_sb[:], bfc.ap())
            nc.sync.dma_start(g_sb[:], g_bc.ap())
            nc.sync.dma_start(b_sb[:], b_bc.ap())

            # ---- Phase 1: vv[tok, hd] = v @ w_v.T + b_v, stored slot-major in
            #      65-col blocks per head (64 values + a ones column).
            with tc.tile_pool(name="p1sb", bufs=1) as p1, \
                 tc.tile_pool(name="p1ps", bufs=2, space="PSUM") as ps1:
                vT_sb = [p1.tile([128, L], BF16, tag=f"vT{i}", name=f"vT{i}") for i in range(8)]
                wv_sb = [p1.tile([128, H * DV], BF16, tag=f"wv{i}", name=f"wv{i}") for i in range(8)]
                for i in range(8):
                    nc.sync.dma_start(vT_sb[i][:], vT.ap()[i * 128:(i + 1) * 128, :])
                    nc.sync.dma_start(wv_sb[i][:], wvT.ap()[i * 128:(i + 1) * 128, :])
                for tokt in range(8):
                    pv = ps1.tile([128, H * DV], F32, tag="pv")
                    for dmt in range(8):
                        for n in range(2):
                            nc.tensor.matmul(
                                pv[:, n * 512:(n + 1) * 512],
                                vT_sb[dmt][:, tokt * 128:(tokt + 1) * 128],
                                wv_sb[dmt][:, n * 512:(n + 1) * 512],
                                start=(dmt == 0), stop=False,
                            )
                    for n in range(2):  # + b_v broadcast over tokens
                        nc.tensor.matmul(
                            pv[:, n * 512:(n + 1) * 512],
                            onb_sb[:],
                            bv_sb[:, n * 512:(n + 1) * 512],
                            start=False, stop=True,
                        )
                    src = pv.rearrange("p (h d) -> p h d", d=DV)
                    dst = vv_sb[tokt].rearrange("p (h c) -> p h c", c=65)
                    nc.vector.tensor_copy(dst[:, :, 0:DV], src)
                    nc.vector.memset(dst[:, :, DV], 1.0)

            # ---- Phase 2: per head: E^T = exp(masked posi^T); einsum + denom;
            #      normalize; owned heads (slots 0,1) also emit A^T in fp32.
            with tc.tile_pool(name="p2sb", bufs=2) as p2, \
                 tc.tile_pool(name="p2own", bufs=2) as p2o, \
                 tc.tile_pool(name="p2ps", bufs=2, space="PSUM") as ps2:
                for h in range(H):
                    stage = p2.tile([128, 8 * L], BF16, tag="stage")
                    for kt in range(8):
                        nc.sync.dma_start(
                            stage[:, kt * L:(kt + 1) * L],
                            posiT.ap()[h, kt * 128:(kt + 1) * 128, :],
                        )
                    for kt in range(8):
                        sl = stage[:, kt * L + kt * 128: kt * L + kt * 128 + 128]
                        nc.vector.tensor_tensor(sl, sl, eye_sb[:], ALU.add)
                    E = p2.tile([128, 8 * L], BF16, tag="E")
                    nc.scalar.activation(E[:], stage[:], AF.Exp)

                    pe = ps2.tile([65, L], F32, tag="pe")
                    for kt in range(8):
                        for n in range(2):
                            nc.tensor.matmul(
                                pe[:, n * 512:(n + 1) * 512],
                                vv_sb[kt][:, h * 65:(h + 1) * 65],
                                E[:, kt * L + n * 512: kt * L + n * 512 + 512],
                                start=(kt == 0), stop=(kt == 7),
                            )
                    rc = p2.tile([1, L], F32, tag="rc")
                    nc.vector.reciprocal(rc[:], pe[64:65, :])
                    pb = ps2.tile([128, L], F32, tag="pb")
                    for n in range(2):
                        nc.tensor.matmul(
                            pb[:, n * 512:(n + 1) * 512],
                            onf_sb[:], rc[:, n * 512:(n + 1) * 512],
                            start=True, stop=True,
                        )
                    bc = p2.tile([128, L], F32, tag="bc")
                    nc.vector.tensor_copy(bc[:], pb[:])

                    r0 = (h % 2) * 64
                    nc.vector.tensor_tensor(
                        outT_sb[h // 2][r0:r0 + 64, :], pe[0:64, :],
                        bc[r0:r0 + 64, :], ALU.mult,
                    )
                    if h < 2:
                        for kt in range(8):
                            af = p2o.tile([128, L], F32, tag="af")
                            nc.scalar.activation(
                                af[:], stage[:, kt * L:(kt + 1) * L], AF.Exp)
                            aa = p2o.tile([128, L], F32, tag="aa")
                            nc.vector.tensor_tensor(aa[:], af[:], bc[:], ALU.mult)
                            nc.sync.dma_start(
                                A_out.ap()[h, kt * 128:(kt + 1) * 128, :], aa[:])

            # ---- Phase 3: fc + bias + LayerNorm.
            with tc.tile_pool(name="p3sb", bufs=2) as p3, \
                 tc.tile_pool(name="p3s1", bufs=2) as p3s, \
                 tc.tile_pool(name="p3ps", bufs=2, space="PSUM") as ps3:
                for qt in range(8):
                    pf = ps3.tile([128, DM], F32, tag="pf")
                    for hdt in range(8):
                        for n in range(2):
                            nc.tensor.matmul(
                                pf[:, n * 512:(n + 1) * 512],
                                outT_sb[hdt][:, qt * 128:(qt + 1) * 128],
                                wfc_sb[hdt][:, n * 512:(n + 1) * 512],
                                start=(hdt == 0), stop=False,
                            )
                    for n in range(2):  # + b_fc
                        nc.tensor.matmul(
                            pf[:, n * 512:(n + 1) * 512],
                            onb_sb[:], bfc_sb[:, n * 512:(n + 1) * 512],
                            start=False, stop=True,
                        )
                    sm = p3s.tile([128, 1], F32, tag="sm")
                    nc.vector.reduce_sum(sm[:], pf[:], axis=AX.X)
                    dummy = p3.tile([128, DM], F32, tag="dummy")
                    sq = p3s.tile([128, 1], F32, tag="sq")
                    nc.scalar.activation(dummy[:], pf[:], AF.Square, accum_out=sq[:])
                    nmu = p3s.tile([128, 1], F32, tag="nmu")
                    nc.vector.tensor_scalar_mul(nmu[:], sm[:], -1.0 / DM)
                    musq = p3s.tile([128, 1], F32, tag="musq")
                    nc.vector.tensor_tensor(musq[:], nmu[:], nmu[:], ALU.mult)
                    var = p3s.tile([128, 1], F32, tag="var")
                    nc.vector.tensor_scalar(var[:], sq[:], 1.0 / DM, LN_EPS,
                                            ALU.mult, ALU.add)
                    nc.vector.tensor_sub(var[:], var[:], musq[:])
                    srt = p3s.tile([128, 1], F32, tag="srt")
                    nc.scalar.activation(srt[:], var[:], AF.Sqrt)
                    r = p3s.tile([128, 1], F32, tag="r")
                    nc.vector.reciprocal(r[:], srt[:])
                    for _ in range(2):  # Newton: r <- r * (1.5 - 0.5 * var * r^2)
                        t = p3s.tile([128, 1], F32, tag="t")
                        nc.vector.tensor_tensor(t[:], r[:], r[:], ALU.mult)
                        nc.vector.tensor_tensor(t[:], t[:], var[:], ALU.mult)
                        nc.vector.tensor_scalar(t[:], t[:], -0.5, 1.5,
                                                ALU.mult, ALU.add)
                        nc.vector.tensor_tensor(r[:], r[:], t[:], ALU.mult)
                    yt = p3.tile([128, DM], F32, tag="yt")
                    nc.vector.tensor_scalar(yt[:], pf[:], nmu[:], r[:],
                                            ALU.add, ALU.mult)
                    nc.vector.tensor_tensor(yt[:], yt[:], g_sb[:], ALU.mult)
                    nc.vector.tensor_tensor(yt[:], yt[:], b_sb[:], ALU.add)
                    nc.sync.dma_start(y_out.ap()[qt * 128:(qt + 1) * 128, :], yt[:])

    nc.compile()
    return nc


def _get_nc():
    if "nc" not in _NC_CACHE:
        _NC_CACHE["nc"] = _build_nc()
    return _NC_CACHE["nc"]


def make_in_maps(v, posi_att, w_v, b_v, w_fc, b_fc, ln_g, ln_b):
    v = np.asarray(v, np.float32)
    posi = np.asarray(posi_att, np.float32)
    w_v = np.asarray(w_v, np.float32)
    b_v = np.asarray(b_v, np.float32)
    w_fc = np.asarray(w_fc, np.float32)
    b_fc = np.asarray(b_fc, np.float32)
    ln_g = np.asarray(ln_g, np.float32)
    ln_b = np.asarray(ln_b, np.float32)

    posiT_all = posi.transpose(0, 2, 1).astype(BF)           # [H, k, q]
    wvT_full = w_v.T.astype(BF)                              # [DM, H*DV]
    wfcT_full = w_fc.T.astype(BF)                            # [H*DV, DM]
    g_bc = np.ascontiguousarray(np.broadcast_to(ln_g, (128, DM)), np.float32)
    b_bc = np.ascontiguousarray(np.broadcast_to(ln_b, (128, DM)), np.float32)
    eye = (np.eye(128, dtype=np.float32) * MASK).astype(BF)
    ones_f = np.ones((1, 128), np.float32)
    ones_b = np.ones((1, 128), BF)
    bfc_r = b_fc.reshape(1, DM).astype(BF)

    in_maps = []
    for c in range(NCORES):
        perm = [2 * c, 2 * c + 1] + [h for h in range(H) if h // 2 != c]
        colidx = np.concatenate([np.arange(p * DV, (p + 1) * DV) for p in perm])
        in_maps.append({
            "posiT": np.ascontiguousarray(posiT_all[perm]),
            "vT": np.ascontiguousarray(v[c].T).astype(BF),
            "wvT": np.ascontiguousarray(wvT_full[:, colidx]),
            "wfcT": np.ascontiguousarray(wfcT_full[colidx, :]),
            "bv": np.ascontiguousarray(b_v[colidx]).reshape(1, H * DV).astype(BF),
            "bfc": bfc_r,
            "g_bc": g_bc,
            "b_bc": b_bc,
            "eye": eye,
            "ones_f": ones_f,
            "ones_b": ones_b,
        })
    return in_maps


def assemble_outputs(results):
    out = np.stack([np.asarray(results[c]["y_out"]) for c in range(NCORES)])
    attn_h = np.empty((H, L, L), np.float32)
    for c in range(NCORES):
        attn_h[2 * c] = results[c]["A_out"][0].T
        attn_h[2 * c + 1] = results[c]["A_out"][1].T
    attn = np.broadcast_to(attn_h[:, None], (H, B, L, L)).reshape(H * B, L, L)
    return out, attn


def kernel(q=None, k=None, v=None, posi_att=None, w_k=None, b_k=None,
           w_v=None, b_v=None, w_fc=None, b_fc=None, ln_g=None, ln_b=None, **_):
    nc = _get_nc()
    in_maps = make_in_maps(v, posi_att, w_v, b_v, w_fc, b_fc, ln_g, ln_b)
    res = run_bass_kernel_spmd(nc, in_maps, core_ids=list(range(NCORES)))
    return assemble_outputs(res.results)


# revision 13
# speedup vs baseline: 1.0535x; 1.0535x over previous
"""Trainium2 Bass kernel for nn_MultiHeadAttention_firstlayer.

Math notes (derived from the reference):
  - d_k == 1 and the key projection contributes kk[b,i,h] uniformly to every
    element of softmax row (h,b,i,:).  Softmax is shift-invariant, so the
    attention probabilities are exactly softmax(mask_diag(posi_att[h])) --
    independent of k, q, w_k, b_k AND of the batch index.
  - Therefore: attn[h*B+b] = A[h] = softmax_row(posi_att[h] masked at the
    diagonal), and out[b] = LN(concat_h(A[h] @ vv[b,:,h,:]) @ w_fc.T + b_fc).

Distribution (8 cores, no collectives):
  - Core c computes the full forward for batch b=c (all 16 heads), and
    additionally writes the normalized attention matrices for heads 2c, 2c+1.
  - All per-core variation is pushed into the input data via a head
    permutation (owned heads first), so one SPMD program serves all cores.

Layouts (host pre-transposes so every matmul contracts over partitions; the
diagonal mask value is pre-applied to posiT on the host):
  - posiT[h, k, q] = posi_att[h, q, k], with posiT[h, i, i] = -30   (bf16)
  - vT[dm, tok] = v[c].T                 (bf16)
  - wvT = w_v.T (col-permuted), wfcT = w_fc.T (row-permuted)   (bf16)
  - einsum:  pe[d, q]  = sum_k vv[k, d] * E^T[k, q]   (ones column gives the
    softmax denominator s[q] as psum row 64 for free)
  - fc:      pf[q, dm] = sum_hd outT[hd, q] * wfcT[hd, dm]
"""

import sys

sys.path.insert(0, "/opt/trn_rl_repo")

import numpy as np
import ml_dtypes

import concourse.mybir as mybir
import concourse.tile as tile
from concourse import bacc
from concourse.bass_utils import run_bass_kernel_spmd

BF = ml_dtypes.bfloat16
F32 = mybir.dt.float32
BF16 = mybir.dt.bfloat16
AF = mybir.ActivationFunctionType
ALU = mybir.AluOpType
AX = mybir.AxisListType

H, DV, DM, B, L = 16, 64, 1024, 8, 1024
NCORES = 8
LN_EPS = 1e-5
MASK = -30.0  # exp(-30) ~ 9e-14: negligible vs row sums ~1e3

_NC_CACHE = {}


def _build_nc():
    nc = bacc.Bacc("TRN2", target_bir_lowering=False, debug=False, num_devices=NCORES)

    posiT = nc.dram_tensor("posiT", [H, L, L], BF16, kind="ExternalInput")
    vT = nc.dram_tensor("vT", [DM, L], BF16, kind="ExternalInput")
    wvT = nc.dram_tensor("wvT", [DM, H * DV], BF16, kind="ExternalInput")
    wfcT = nc.dram_tensor("wfcT", [H * DV, DM], BF16, kind="ExternalInput")
    bv = nc.dram_tensor("bv", [1, H * DV], BF16, kind="ExternalInput")
    bfc = nc.dram_tensor("bfc", [1, DM], BF16, kind="ExternalInput")
    g_bc = nc.dram_tensor("g_bc", [128, DM], F32, kind="ExternalInput")
    b_bc = nc.dram_tensor("b_bc", [128, DM], F32, kind="ExternalInput")

    A_out = nc.dram_tensor("A_out", [2, L, L], F32, kind="ExternalOutput")
    y_out = nc.dram_tensor("y_out", [L, DM], F32, kind="ExternalOutput")

    with tile.TileContext(nc) as tc:
        with tc.tile_pool(name="const", bufs=1) as cp, \
             tc.tile_pool(name="work", bufs=2) as wp, \
             tc.tile_pool(name="ln", bufs=2) as lp, \
             tc.tile_pool(name="ps", bufs=2, space="PSUM") as ps:
            wfc_sb = cp.tile([128, 8 * DM], BF16, tag="wfcall", name="wfcall")
            vv_sb = [cp.tile([128, H * 65], BF16, tag=f"vv{i}", name=f"vv{i}")
                     for i in range(8)]
            outT_sb = [cp.tile([128, L], BF16, tag=f"oT{i}", name=f"oT{i}")
                       for i in range(8)]
            vT_sb = cp.tile([128, 8 * L], BF16, tag="vTall", name="vTall")
            wv_sb = cp.tile([128, 8 * H * DV], BF16, tag="wvall", name="wvall")
            onb_sb = cp.tile([1, 128], BF16, tag="onb")
            bv_sb = cp.tile([1, H * DV], BF16, tag="bv")
            bfc_sb = cp.tile([1, DM], BF16, tag="bfc")
            g_sb = cp.tile([128, DM], F32, tag="g")
            b_sb = cp.tile([128, DM], F32, tag="b")

            nc.vector.memset(onb_sb[:], 1.0)
            nc.sync.dma_start(bv_sb[:], bv.ap())
            nc.sync.dma_start(bfc_sb[:], bfc.ap())
            nc.sync.dma_start(g_sb[:], g_bc.ap())
            nc.sync.dma_start(b_sb[:], b_bc.ap())
            for i in range(8):
                nc.sync.dma_start(vT_sb[:, i * L:(i + 1) * L],
                                  vT.ap()[i * 128:(i + 1) * 128, :])
                nc.sync.dma_start(wv_sb[:, i * H * DV:(i + 1) * H * DV],
                                  wvT.ap()[i * 128:(i + 1) * 128, :])
            nc.sync.dma_start(wfc_sb.rearrange("p (i q) -> p i q", q=DM),
                              wfcT.ap().rearrange("(i p) q -> p i q", p=128))

            # ---- Phase 1: vv[tok, hd] = v @ w_v.T + b_v, stored slot-major in
            #      65-col blocks per head (64 values + a ones column).
            for tokt in range(8):
                pv = ps.tile([128, H * DV], F32, tag="acc", name=f"pv{tokt}")
                for dmt in range(8):
                    for n in range(2):
                        nc.tensor.matmul(
                            pv[:, n * 512:(n + 1) * 512],
                            vT_sb[:, dmt * L + tokt * 128: dmt * L + (tokt + 1) * 128],
                            wv_sb[:, dmt * H * DV + n * 512: dmt * H * DV + (n + 1) * 512],
                            start=(dmt == 0), stop=False,
                        )
                for n in range(2):  # + b_v broadcast over tokens
                    nc.tensor.matmul(
                        pv[:, n * 512:(n + 1) * 512],
                        onb_sb[:],
                        bv_sb[:, n * 512:(n + 1) * 512],
                        start=False, stop=True,
                    )
                src = pv.rearrange("p (h d) -> p h d", d=DV)
                dst = vv_sb[tokt].rearrange("p (h c) -> p h c", c=65)
                nc.vector.tensor_copy(dst[:, :, 0:DV], src)
                nc.vector.memset(dst[:, :, DV], 1.0)

            # ---- Phase 2: per head: E^T = exp(posiT) in-place; einsum + denom;
            #      normalize outT; owned heads (slots 7, 15) emit A^T in fp32.
            #      Slot 7's A-normalizations are deferred and emitted in halves
            #      at the top of heads 9 and 10 so the DVE work doesn't stall
            #      the reciprocal->broadcast->normalize chain of adjacent heads.
            own_pending = []

            def emit_own_half(oh, ostage, obc, half):
                aa = wp.tile([128, 4 * L], F32, tag="aa", name=f"aa{oh}_{half}")
                for j in range(4):
                    kt = half * 4 + j
                    nc.vector.tensor_tensor(
                        aa[:, j * L:(j + 1) * L],
                        ostage[:, kt * L:(kt + 1) * L], obc[:], ALU.mult)
                nc.gpsimd.dma_start(
                    A_out.ap()[oh // 8, half * 512:(half + 1) * 512, :]
                         .rearrange("(kt p) q -> p kt q", p=128),
                    aa.rearrange("p (kt q) -> p kt q", q=L))

            for h in range(H):
                if h in (9, 10) and own_pending:
                    oh, ostage, obc = own_pending[0]
                    emit_own_half(oh, ostage, obc, h - 9)
                    if h == 10:
                        own_pending.pop(0)
                stage = wp.tile([128, 8 * L], BF16, tag="stage", name=f"st{h}",
                                bufs=3)
                nc.sync.dma_start(
                    stage.rearrange("p (kt q) -> p kt q", q=L),
                    posiT.ap()[h].rearrange("(kt p) q -> p kt q", p=128),
                )
                for c2 in range(2):  # exp in 2 chunks of 4096 for pipelining
                    nc.scalar.activation(
                        stage[:, c2 * 4096:(c2 + 1) * 4096],
                        stage[:, c2 * 4096:(c2 + 1) * 4096], AF.Exp)
                pe = ps.tile([65, L], F32, tag="pe", name=f"pe{h}")
                for kt in range(8):
                    for n in range(2):
                        nc.tensor.matmul(
                            pe[:, n * 512:(n + 1) * 512],
                            vv_sb[kt][:, h * 65:(h + 1) * 65],
                            stage[:, kt * L + n * 512: kt * L + n * 512 + 512],
                            start=(kt == 0), stop=(kt == 7),
                        )
                rc = wp.tile([1, L], F32, tag="rc", name=f"rc{h}")
                nc.vector.reciprocal(rc[:], pe[64:65, :])
                bc = wp.tile([128, L], F32, tag="bc", name=f"bc{h}")
                nc.gpsimd.partition_broadcast(bc[:], rc[:])

                r0 = (h % 2) * 64
                nc.vector.tensor_tensor(
                    outT_sb[h // 2][r0:r0 + 64, :], pe[0:64, :],
                    bc[r0:r0 + 64, :], ALU.mult,
                )
                if h in (7, 15):
                    own_pending.append((h, stage, bc))
            # slot 15's A output at the tail (overlaps the fc matmuls)
            oh, ostage, obc = own_pending.pop(0)
            emit_own_half(oh, ostage, obc, 0)
            emit_own_half(oh, ostage, obc, 1)

            # ---- Phase 3: fc + bias + LayerNorm (LN chain pipelined one
            #      qt behind the matmuls so DVE never head-of-line blocks).
            def ln_chain(qt, yt):
                sm = lp.tile([128, 1], F32, tag="sm", name=f"sm{qt}")
                nc.vector.reduce_sum(sm[:], yt[:], axis=AX.X)
                dummy = lp.tile([128, DM], F32, tag="dummy", name=f"dm{qt}",
                                bufs=1)
                sq = lp.tile([128, 1], F32, tag="sq", name=f"sq{qt}")
                nc.scalar.activation(dummy[:], yt[:], AF.Square, accum_out=sq[:])
                nmu = lp.tile([128, 1], F32, tag="nmu", name=f"nmu{qt}")
                nc.vector.tensor_scalar_mul(nmu[:], sm[:], -1.0 / DM)
                musq = lp.tile([128, 1], F32, tag="musq", name=f"musq{qt}")
                nc.vector.tensor_tensor(musq[:], nmu[:], nmu[:], ALU.mult)
                var = lp.tile([128, 1], F32, tag="var", name=f"var{qt}")
                nc.vector.tensor_scalar(var[:], sq[:], 1.0 / DM, LN_EPS,
                                        ALU.mult, ALU.add)
                nc.vector.tensor_sub(var[:], var[:], musq[:])
                srt = lp.tile([128, 1], F32, tag="srt", name=f"srt{qt}")
                nc.scalar.activation(srt[:], var[:], AF.Sqrt)
                r = lp.tile([128, 1], F32, tag="r", name=f"r{qt}")
                nc.vector.reciprocal(r[:], srt[:])
                t = lp.tile([128, 1], F32, tag="t", name=f"t{qt}")
                nc.vector.tensor_tensor(t[:], r[:], r[:], ALU.mult)
                nc.vector.tensor_tensor(t[:], t[:], var[:], ALU.mult)
                nc.vector.tensor_scalar(t[:], t[:], -0.5, 1.5, ALU.mult, ALU.add)
                nc.vector.tensor_tensor(r[:], r[:], t[:], ALU.mult)
                nc.vector.tensor_scalar(yt[:], yt[:], nmu[:], r[:],
                                        ALU.add, ALU.mult)
                nc.vector.tensor_tensor(yt[:], yt[:], g_sb[:], ALU.mult)
                nc.vector.tensor_tensor(yt[:], yt[:], b_sb[:], ALU.add)
                nc.gpsimd.dma_start(y_out.ap()[qt * 128:(qt + 1) * 128, :], yt[:])

            pending = None
            for qt in range(8):
                pf = ps.tile([128, DM], F32, tag="acc", name=f"pf{qt}")
                for hdt in range(8):
                    for n in range(2):
                        nc.tensor.matmul(
                            pf[:, n * 512:(n + 1) * 512],
                            outT_sb[hdt][:, qt * 128:(qt + 1) * 128],
                            wfc_sb[:, hdt * DM + n * 512: hdt * DM + (n + 1) * 512],
                            start=(hdt == 0), stop=False,
                        )
                for n in range(2):  # + b_fc
                    nc.tensor.matmul(
                        pf[:, n * 512:(n + 1) * 512],
                        onb_sb[:], bfc_sb[:, n * 512:(n + 1) * 512],
                        start=False, stop=True,
                    )
                yt = lp.tile([128, DM], F32, tag="yt", name=f"yt{qt}", bufs=3)
                nc.vector.tensor_copy(yt[:], pf[:])
                if pending is not None:
                    ln_chain(*pending)
                pending = (qt, yt)
            ln_chain(*pending)

    nc.compile()
    return nc


def _get_nc():
    if "nc" not in _NC_CACHE:
        _NC_CACHE["nc"] = _build_nc()
    return _NC_CACHE["nc"]


def make_in_maps(v, posi_att, w_v, b_v, w_fc, b_fc, ln_g, ln_b):
    v = np.asarray(v, np.float32)
    posi = np.asarray(posi_att, np.float32)
    w_v = np.asarray(w_v, np.float32)
    b_v = np.asarray(b_v, np.float32)
    w_fc = np.asarray(w_fc, np.float32)
    b_fc = np.asarray(b_fc, np.float32)
    ln_g = np.asarray(ln_g, np.float32)
    ln_b = np.asarray(ln_b, np.float32)

    posiT_all = posi.transpose(0, 2, 1).astype(BF)           # [H, k, q]
    ii = np.arange(L)
    posiT_all[:, ii, ii] = np.float32(MASK)                  # diagonal mask
    wvT_full = w_v.T.astype(BF)                              # [DM, H*DV]
    wfcT_full = w_fc.T.astype(BF)                            # [H*DV, DM]
    g_bc = np.ascontiguousarray(np.broadcast_to(ln_g, (128, DM)), np.float32)
    b_bc = np.ascontiguousarray(np.broadcast_to(ln_b, (128, DM)), np.float32)
    bfc_r = b_fc.reshape(1, DM).astype(BF)

    in_maps = []
    for c in range(NCORES):
        others = [h for h in range(H) if h // 2 != c]
        perm = others[:7] + [2 * c] + others[7:] + [2 * c + 1]
        colidx = np.concatenate([np.arange(p * DV, (p + 1) * DV) for p in perm])
        in_maps.append({
            "posiT": np.ascontiguousarray(posiT_all[perm]),
            "vT": np.ascontiguousarray(v[c].T).astype(BF),
            "wvT": np.ascontiguousarray(wvT_full[:, colidx]),
            "wfcT": np.ascontiguousarray(wfcT_full[colidx, :]),
            "bv": np.ascontiguousarray(b_v[colidx]).reshape(1, H * DV).astype(BF),
            "bfc": bfc_r,
            "g_bc": g_bc,
            "b_bc": b_bc,
        })
    return in_maps


def assemble_outputs(results):
    out = np.stack([np.asarray(results[c]["y_out"]) for c in range(NCORES)])
    attn_h = np.empty((H, L, L), np.float32)
    for c in range(NCORES):
        attn_h[2 * c] = results[c]["A_out"][0].T
        attn_h[2 * c + 1] = results[c]["A_out"][1].T
    attn = np.broadcast_to(attn_h[:, None], (H, B, L, L)).reshape(H * B, L, L)
    return out, attn


def kernel(q=None, k=None, v=None, posi_att=None, w_k=None, b_k=None,
           w_v=None, b_v=None, w_fc=None, b_fc=None, ln_g=None, ln_b=None, **_):
    nc = _get_nc()
    in_maps = make_in_maps(v, posi_att, w_v, b_v, w_fc, b_fc, ln_g, ln_b)
    last_err = None
    for _attempt in range(4):
        try:
            res = run_bass_kernel_spmd(nc, in_maps, core_ids=list(range(NCORES)))
            return assemble_outputs(res.results)
        except Exception as e:  # transient backend execution errors
            last_err = e
            import time as _time
            _time.sleep(5.0)
    raise last_err
